# revision 1
# baseline (speedup 1.0000x reference)
"""Trainium2 Bass kernel for the RGCN message-passing model (nn_Actor_12094627905962).

Strategy (8 NeuronCores, dst-sharded):
  - Each core owns a contiguous range of NS=12500 destination nodes and all
    edges pointing into them (edge counts are near-balanced).
  - Per-relation node transforms (N @ W1[r,:64], H @ W2[r]) are computed on the
    owning core's node shard and AllGathered into per-core DRAM tables, so the
    per-edge work becomes pure row gathers.
  - Layer-1 sum aggregation: multi-pass indirect DMA gathers with CCE add
    accumulating into SBUF bank tiles (passes are prefix slices of a
    degree-ranked node ordering -> no wasted reads, no RMW races).
  - The edge-feature term relu(ea@We+be)@W1[r,64:96] is computed densely in
    rel-sorted order into a DRAM table and folded into the same accumulation.
  - Layer-2 per-(dst,rel) max aggregation: bypass gathers into staging tiles +
    DVE elementwise max folds; empty groups never appear (only nonempty
    (dst,rel) groups get accumulator slots).
  - Head + encoders are small dense matmuls on the node shard.

All float math runs on device; the host side only shards/permutes inputs and
converts the integer edge list into gather index tables (u32 row ids).
"""

import sys

if "/opt/trn_rl_repo" not in sys.path:
    sys.path.insert(0, "/opt/trn_rl_repo")

import numpy as np
import ml_dtypes

BF = ml_dtypes.bfloat16

N = 100_000
E = 1_600_000
R = 8
NCORES = 8
NS = N // NCORES  # 12500
D = 64
NSP = 12800  # node positions padded (100 chunks of 128, 25 blocks of 512)
NCH = NSP // 128  # 100
SEG = R * NSP + 2  # rows per rank segment in gathered tables (+zero,+neg rows)
ZROW = R * NSP  # zero row offset within a segment
NROW = R * NSP + 1  # -big row offset within a segment
NEGBIG = -1.0e30
L2_SLAB_CHUNKS = 120  # chunks per layer-2 slab


# ---------------------------------------------------------------------------
# host-side preprocessing
# ---------------------------------------------------------------------------


def _ceil(a, b):
    return -(-a // b)


def preprocess(edge_index, edge_type):
    """Pure index preprocessing. Returns per-core structures + common sizes."""
    src = np.asarray(edge_index[0], np.int64)
    dst = np.asarray(edge_index[1], np.int64)
    rel = np.asarray(edge_type, np.int64)

    core_of_edge = dst // NS
    cores = []
    for c in range(NCORES):
        m = np.nonzero(core_of_edge == c)[0]
        cores.append(
            {
                "eids": m,
                "s": src[m],
                "d": dst[m] - c * NS,
                "r": rel[m],
            }
        )

    # global rank: per core, nodes ordered by layer-1 in-degree (desc)
    grank = np.empty(N, np.int64)
    for c in range(NCORES):
        deg = np.bincount(cores[c]["d"], minlength=NS)
        order = np.argsort(-deg, kind="stable")
        rank = np.empty(NS, np.int64)
        rank[order] = np.arange(NS)
        cores[c]["deg"] = deg
        cores[c]["rank"] = rank  # node -> position
        grank[c * NS : (c + 1) * NS] = rank + 0

    for c in range(NCORES):
        cc = cores[c]
        s, d, r = cc["s"], cc["d"], cc["r"]
        ne = len(s)
        rank = cc["rank"]

        # --- rel-sorted layout for the e-term table ---
        rorder = np.argsort(r, kind="stable")
        cc["rorder"] = rorder
        cc["rcnt"] = np.bincount(r, minlength=R)

        # --- L1: per-node edge slot (j-th edge of its dst) ---
        dorder = np.argsort(d, kind="stable")
        ds = d[dorder]
        starts = np.searchsorted(ds, np.arange(NS))
        j1 = np.arange(ne) - starts[ds]
        l1pos = rank[ds]  # accumulator position of each (sorted) edge
        cc["l1_edge"] = dorder  # edge order for l1 passes
        cc["l1_j"] = j1
        cc["l1_pos"] = l1pos
        cc["maxd1"] = int(cc["deg"].max()) if ne else 0
        # count of nodes with deg >= j+1 (pass sizes)
        degsorted = -np.sort(-cc["deg"])
        cc["cnt1"] = np.array(
            [int((degsorted >= j + 1).sum()) for j in range(cc["maxd1"])], np.int64
        )

        # --- L2: (dst, rel) groups ---
        g = d * R + r
        gorder = np.argsort(g, kind="stable")
        gs = g[gorder]
        uniq, uidx, ucnt = np.unique(gs, return_index=True, return_counts=True)
        ngroups = len(uniq)
        grank2 = np.argsort(-ucnt, kind="stable")
        slot_of_u = np.empty(ngroups, np.int64)
        slot_of_u[grank2] = np.arange(ngroups)
        # edge -> its group's slot and j index
        gid_of_edge = np.searchsorted(uniq, gs)  # for edges in gorder
        j2 = np.arange(ne) - uidx[gid_of_edge]
        cc["l2_edge"] = gorder
        cc["l2_j"] = j2
        cc["l2_slot"] = slot_of_u[gid_of_edge]
        cc["l2_ngroups"] = ngroups
        cc["maxd2"] = int(ucnt.max()) if ne else 0
        csorted = -np.sort(-ucnt)
        cc["cnt2"] = np.array(
            [int((csorted >= j + 1).sum()) for j in range(cc["maxd2"])], np.int64
        )
        # stage2: for each dst node, list of its nonempty-rel slots
        cc["s2_dst"] = uniq // R  # dst of each group (by uid)
        cc["s2_slot"] = slot_of_u  # slot of each group (by uid)

    # ---- common (max-over-cores) sizes ----
    maxd1 = max(c["maxd1"] for c in cores)
    maxd2 = max(c["maxd2"] for c in cores)
    cmax1 = np.zeros(maxd1, np.int64)
    cmax2 = np.zeros(maxd2, np.int64)
    for c in cores:
        cmax1[: c["maxd1"]] = np.maximum(cmax1[: c["maxd1"]], c["cnt1"])
        cmax2[: c["maxd2"]] = np.maximum(cmax2[: c["maxd2"]], c["cnt2"])
    C1 = np.array([_ceil(int(x), 128) for x in cmax1], np.int64)  # chunks per l1 pass
    SLOTMAX = max(c["l2_ngroups"] for c in cores)
    CH2 = _ceil(SLOTMAX, 128)  # total l2 chunks
    C2 = np.array([_ceil(int(x), 128) for x in cmax2], np.int64)
    C2[0] = CH2  # pass 0 (bypass) covers everything incl. dummy slots
    # e-term table: per-rel block sizes (common), padded to 2048
    rcmax = np.zeros(R, np.int64)
    for c in cores:
        rcmax = np.maximum(rcmax, c["rcnt"])
    eblk = np.array([_ceil(int(x), 2048) * 2048 for x in rcmax], np.int64)
    eoff = np.concatenate([[0], np.cumsum(eblk)])
    ETOT = int(eoff[-1]) + 1  # +1 zero row
    EPAD = int(eoff[-1])

    common = {
        "maxd1": maxd1,
        "maxd2": maxd2,
        "C1": C1,
        "C2": C2,
        "CH2": CH2,
        "SLOTMAX": SLOTMAX,
        "eblk": eblk,
        "eoff": eoff,
        "ETOT": ETOT,
        "EPAD": EPAD,
        "grank": grank,
    }

    # ---- per-core index arenas ----
    # column layout: [L1e passes | L1n passes | L2 passes | stage2 8 passes]
    ncol_l1 = int(C1.sum())
    ncol_l2 = int(C2.sum())
    TC = 2 * ncol_l1 + ncol_l2 + 8 * NCH
    common["TC"] = TC
    col_l1e = np.concatenate([[0], np.cumsum(C1)])[:-1]
    col_l1n = col_l1e + ncol_l1
    col_l2 = np.concatenate([[0], np.cumsum(C2)])[:-1] + 2 * ncol_l1
    col_s2 = 2 * ncol_l1 + ncol_l2 + np.arange(8) * NCH
    common["col_l1e"] = col_l1e
    common["col_l1n"] = col_l1n
    common["col_l2"] = col_l2
    common["col_s2"] = col_s2

    for c in range(NCORES):
        cc = cores[c]
        s, d, r = cc["s"], cc["d"], cc["r"]
        arena = np.zeros((128, TC), np.uint32)

        # table row of (rel, src) in the gathered node tables
        def _ntab_row(rr, ss):
            return (ss // NS) * SEG + rr * NSP + grank[ss]

        zdummy = np.uint32(ZROW)  # core0 segment zero row
        ndummy = np.uint32(NROW)  # core0 segment -big row

        # Etable row of each edge (rel-sorted layout)
        erow = np.empty(len(s), np.int64)
        ro = cc["rorder"]
        pos_in_rel = np.zeros(len(s), np.int64)
        off = 0
        for rr in range(R):
            cnt = int(cc["rcnt"][rr])
            pos_in_rel[ro[off : off + cnt]] = np.arange(cnt) + int(eoff[rr])
            off += cnt
        erow[:] = pos_in_rel

        def _fill(colbase, nchunks, positions, rows, dummy):
            block = np.full(nchunks * 128, dummy, np.uint32)
            block[positions] = rows.astype(np.uint32)
            arena[:, colbase : colbase + nchunks] = block.reshape(nchunks, 128).T

        # L1 passes
        le, lj, lp = cc["l1_edge"], cc["l1_j"], cc["l1_pos"]
        for j in range(maxd1):
            m = lj == j
            if j < len(C1):
                nchunk = int(C1[j])
            pos = lp[m]
            eids = le[m]
            _fill(int(col_l1e[j]), nchunk, pos, erow[eids].astype(np.uint32), np.uint32(EPAD))
            nrows = _ntab_row(r[eids], s[eids])
            _fill(int(col_l1n[j]), nchunk, pos, nrows, zdummy)

        # L2 passes
        ge, gj, gslot = cc["l2_edge"], cc["l2_j"], cc["l2_slot"]
        for j in range(maxd2):
            m = gj == j
            nchunk = int(C2[j])
            pos = gslot[m]
            eids = ge[m]
            hrows = _ntab_row(r[eids], s[eids])
            _fill(int(col_l2[j]), nchunk, pos, hrows, ndummy)

        # stage2 passes: node at position p gets its k-th nonempty-rel slot
        s2d = cc["s2_dst"]
        s2slot = cc["s2_slot"]
        rank = cc["rank"]
        # per dst, the list of slots (order by uid == rel order)
        dorder2 = np.argsort(s2d, kind="stable")
        sd = s2d[dorder2]
        st = np.searchsorted(sd, np.arange(NS))
        en = np.searchsorted(sd, np.arange(NS), side="right")
        nrels = en - st
        for k in range(8):
            m = nrels > k
            pos = rank[np.nonzero(m)[0]]
            rows = s2slot[dorder2[st[m] + k]].astype(np.uint32)
            _fill(int(col_s2[k]), NCH, pos, rows, np.uint32(CH2 * 128))

        cc["arena"] = arena

    return cores, common


def build_core_inputs(inputs, cores, common):
    """Per-core numpy input dict for run_bass_kernel_spmd."""
    x = np.asarray(inputs["x"], np.float32)
    ea = np.asarray(inputs["edge_attr"], np.float32)
    om = np.asarray(inputs["omega"], np.float32)

    f = lambda k: np.asarray(inputs[k], np.float32)
    Wn, bn = f("Wn"), f("bn")
    We, be = f("We"), f("be")
    Wo, bo = f("Wo"), f("bo")
    W1, Wroot1, b1 = f("W1"), f("Wroot1"), f("b1")
    W2, Wroot2, b2 = f("W2"), f("Wroot2"), f("b2")
    Wagg, bagg = f("Wagg"), f("bagg")
    Wc, bc = f("Wc"), f("bc")

    # f32 weight pack [10, 64]: Wnx 0:4, Wox 4:7, Wex 7:10 (cols 0:32)
    wf = np.zeros((10, 64), np.float32)
    wf[0:3, :] = Wn
    wf[3, :] = bn
    wf[4:6, :] = Wo
    wf[6, :] = bo
    wf[7:9, :32] = We
    wf[9, :32] = be

    # bf16 pack rows:
    #   0:512   W1a (r*64+k, f)
    #   512:768 M   (r*32+k, f)
    #   768:1280 W2
    #   1280:1345 Wroot1x
    #   1345:1410 Wroot2x
    #   1410:1475 WaggAx
    #   1475:1539 WaggB
    #   1539:1604 Wcx (col 0)
    wb = np.zeros((1604, 64), np.float32)
    wb[0:512] = W1[:, :64, :].reshape(512, 64)
    wb[512:768] = W1[:, 64:96, :].reshape(256, 64)
    wb[768:1280] = W2.reshape(512, 64)
    wb[1280:1344] = Wroot1
    wb[1344] = b1
    wb[1345:1409] = Wroot2
    wb[1409] = b2
    wb[1410:1474] = Wagg[:64]
    wb[1474] = bagg
    wb[1475:1539] = Wagg[64:]
    wb[1539:1603, 0] = Wc[:, 0]
    wb[1603, 0] = bc[0]
    wb = wb.astype(BF)

    EPAD = common["EPAD"]
    in_maps = []
    for c in range(NCORES):
        cc = cores[c]
        rank = cc["rank"]
        inv = np.argsort(rank)  # position -> node
        xT = np.zeros((4, NSP), np.float32)
        xT[:3, :NS] = x[c * NS : (c + 1) * NS][inv].T
        xT[3, :] = 1.0
        omT = np.zeros((3, NSP), np.float32)
        omT[:2, :NS] = om[c * NS : (c + 1) * NS][inv].T
        omT[2, :] = 1.0
        eaT = np.zeros((3, EPAD), np.float32)
        ro = cc["rorder"]
        off = 0
        for rr in range(R):
            cnt = int(cc["rcnt"][rr])
            sl = slice(int(common["eoff"][rr]), int(common["eoff"][rr]) + cnt)
            eaT[:2, sl] = ea[cc["eids"][ro[off : off + cnt]]].T
            off += cnt
        eaT[2, :] = 1.0
        in_maps.append(
            {
                "xT": xT,
                "omT": omT,
                "eaT": eaT,
                "wf": wf,
                "wb": wb,
                "idxs": cc["arena"],
            }
        )
    return in_maps


# ---------------------------------------------------------------------------
# device graph
# ---------------------------------------------------------------------------


def build_graph(common):
    import concourse.bacc as bacc
    import concourse.bass as bass
    import concourse.mybir as mybir
    from concourse.tile import TileContext
    from concourse.masks import make_identity

    fp32 = mybir.dt.float32
    bf16 = mybir.dt.bfloat16
    u32 = mybir.dt.uint32
    AX = mybir.AluOpType

    C1, C2 = common["C1"], common["C2"]
    maxd1, maxd2 = common["maxd1"], common["maxd2"]
    col_l1e, col_l1n = common["col_l1e"], common["col_l1n"]
    col_l2, col_s2 = common["col_l2"], common["col_s2"]
    CH2 = common["CH2"]
    EPAD, ETOT = common["EPAD"], common["ETOT"]
    TC = common["TC"]
    SLOTMAX = common["SLOTMAX"]
    S2ROWS = CH2 * 128 + 1

    nc = bacc.Bacc(None, target_bir_lowering=False)

    xT = nc.dram_tensor("xT", [4, NSP], fp32, kind="ExternalInput")
    omT = nc.dram_tensor("omT", [3, NSP], fp32, kind="ExternalInput")
    eaT = nc.dram_tensor("eaT", [3, EPAD], fp32, kind="ExternalInput")
    wf = nc.dram_tensor("wf", [10, 64], fp32, kind="ExternalInput")
    wb = nc.dram_tensor("wb", [1604, 64], bf16, kind="ExternalInput")
    idxs_d = nc.dram_tensor("idxs", [128, TC], u32, kind="ExternalInput")
    out_d = nc.dram_tensor("out", [128, NCH], fp32, kind="ExternalOutput")

    agN_in = nc.dram_tensor("agN_in", [SEG, 64], bf16)
    agH_in = nc.dram_tensor("agH_in", [SEG, 64], bf16)
    agN_out = nc.dram_tensor("agN_out", [NCORES * SEG, 64], bf16, addr_space="Shared")
    agH_out = nc.dram_tensor("agH_out", [NCORES * SEG, 64], bf16, addr_space="Shared")
    etab = nc.dram_tensor("etab", [ETOT, 64], bf16)
    s2buf = nc.dram_tensor("s2buf", [S2ROWS, 64], bf16)

    groups = [list(range(NCORES))]

    with TileContext(nc) as tc:
        with (
            tc.tile_pool(name="persist", bufs=1) as pp,
            tc.tile_pool(name="psA", bufs=2, space="PSUM") as psA,
            tc.tile_pool(name="psB", bufs=2, space="PSUM") as psB,
            tc.tile_pool(name="psC", bufs=4, space="PSUM") as psC,
            tc.tile_pool(name="work", bufs=3) as wk,
        ):
            # ---- persistent tiles ----
            idxs = pp.tile([128, TC], u32)
            nc.sync.dma_start(out=idxs[:], in_=idxs_d[:, :])

            wnx = pp.tile([4, 64], fp32)
            nc.sync.dma_start(out=wnx[:], in_=wf[0:4, :])
            wox = pp.tile([3, 64], fp32)
            nc.sync.dma_start(out=wox[:], in_=wf[4:7, :])
            wex = pp.tile([3, 32], fp32)
            nc.sync.dma_start(out=wex[:], in_=wf[7:10, 0:32])
            w1a = pp.tile([64, R * 64], bf16)
            wm = pp.tile([32, R * 64], bf16)
            w2a = pp.tile([64, R * 64], bf16)
            for rr in range(R):
                fs = slice(rr * 64, (rr + 1) * 64)
                nc.sync.dma_start(out=w1a[:, fs], in_=wb[rr * 64 : (rr + 1) * 64, :])
                nc.sync.dma_start(
                    out=wm[:, fs], in_=wb[512 + rr * 32 : 512 + (rr + 1) * 32, :]
                )
                nc.sync.dma_start(
                    out=w2a[:, fs], in_=wb[768 + rr * 64 : 768 + (rr + 1) * 64, :]
                )
            wroot1 = pp.tile([65, 64], bf16)
            nc.sync.dma_start(out=wroot1[:], in_=wb[1280:1345, :])
            wroot2 = pp.tile([65, 64], bf16)
            nc.sync.dma_start(out=wroot2[:], in_=wb[1345:1410, :])
            wagga = pp.tile([65, 64], bf16)
            nc.sync.dma_start(out=wagga[:], in_=wb[1410:1475, :])
            waggb = pp.tile([64, 64], bf16)
            nc.sync.dma_start(out=waggb[:], in_=wb[1475:1539, :])
            wcx = pp.tile([65, 64], bf16)
            nc.sync.dma_start(out=wcx[:], in_=wb[1539:1604, :])

            ident = pp.tile([128, 128], bf16)
            make_identity(nc, ident[:])

            # zero / -big rows for table padding
            zrow = pp.tile([1, 64], bf16)
            nc.vector.memset(zrow[:], 0.0)
            nrow = pp.tile([1, 64], bf16)
            nc.vector.memset(nrow[:], NEGBIG)

            # ---- encoders (feature-major) ----
            nT = pp.tile([65, NSP], bf16)
            nc.vector.memset(nT[64:65, :], 1.0)
            for b in range(NSP // 512):
                sl = slice(b * 512, (b + 1) * 512)
                xch = wk.tile([4, 512], fp32, tag="xch")
                nc.sync.dma_start(out=xch[:], in_=xT[:, sl])
                p1 = psA.tile([64, 512], fp32, space="PSUM", tag="pa")
                nc.tensor.matmul(p1[:], lhsT=wnx[:], rhs=xch[:], start=True, stop=True)
                nc.scalar.activation(nT[0:64, sl], p1[:], mybir.ActivationFunctionType.Relu)

            # ---- N-table: per node chunk, all 8 rel transforms in one matmul ----
            for ch in range(NCH):
                sl = slice(ch * 128, (ch + 1) * 128)
                p = psB.tile([128, R * 64], fp32, space="PSUM", tag="p")
                nc.tensor.matmul(
                    p[:], lhsT=nT[0:64, sl], rhs=w1a[:], start=True, stop=True
                )
                stagN = wk.tile([128, R, 64], bf16, tag="stagN")
                nc.scalar.activation(
                    stagN[:].rearrange("p r f -> p (r f)"),
                    p[:],
                    mybir.ActivationFunctionType.Copy,
                )
                # rows r*NSP + ch*128 + p for this chunk
                nc.sync.dma_start(
                    out=agN_in[0 : R * NSP, :]
                    .rearrange("(r ch p) f -> ch p r f", p=128, ch=NCH)[ch],
                    in_=stagN[:],
                )
            nc.sync.dma_start(out=agN_in[ZROW : ZROW + 1, :], in_=zrow[:])
            nc.sync.dma_start(out=agN_in[NROW : NROW + 1, :], in_=nrow[:])
            nc.gpsimd.collective_compute(
                "AllGather",
                mybir.AluOpType.bypass,
                replica_groups=groups,
                ins=[agN_in[:, :]],
                outs=[agN_out[:, :]],
            )

            # ---- e-term table (rel-sorted blocks of 2048 edges) ----
            if True:
                nc.sync.dma_start(out=etab[ETOT - 1 : ETOT, :], in_=zrow[:])
                eblk = common["eblk"]
                eoff = common["eoff"]
                for rr in range(R):
                    for blk in range(int(eblk[rr]) // 2048):
                        base = int(eoff[rr]) + blk * 2048
                        each = wk.tile([3, 2048], fp32, tag="each")
                        nc.sync.dma_start(out=each[:], in_=eaT[:, base : base + 2048])
                        stagE = wk.tile([128, 16, 64], bf16, tag="stagE")
                        for q in range(4):  # 4 x 512 edges
                            sl = slice(q * 512, (q + 1) * 512)
                            pe = psA.tile([32, 512], fp32, space="PSUM", tag="pa")
                            nc.tensor.matmul(
                                pe[:], lhsT=wex[:], rhs=each[:, sl], start=True, stop=True
                            )
                            et = wk.tile([32, 512], bf16, tag="et")
                            nc.scalar.activation(
                                et[:], pe[:], mybir.ActivationFunctionType.Relu
                            )
                            pm = psB.tile([128, 4 * 64], fp32, space="PSUM", tag="p")
                            for u in range(4):  # 4 x 128 edges
                                nc.tensor.matmul(
                                    pm[:, u * 64 : (u + 1) * 64],
                                    lhsT=et[:, u * 128 : (u + 1) * 128],
                                    rhs=wm[:, rr * 64 : (rr + 1) * 64],
                                    start=True,
                                    stop=True,
                                )
                            nc.vector.tensor_copy(
                                stagE[:, q * 4 : (q + 1) * 4, :].rearrange(
                                    "p a f -> p (a f)"
                                ),
                                pm[:],
                            )
                        nc.sync.dma_start(
                            out=etab[base : base + 2048, :].rearrange(
                                "(ch p) f -> p ch f", p=128
                            ),
                            in_=stagE[:],
                        )

            # ---- layer-1 accumulation: per-chunk staged gathers + DVE adds ----
            hT = pp.tile([65, NSP], bf16)
            nc.vector.memset(hT[64:65, :], 1.0)
            acc1 = pp.tile([128, NCH, 64], bf16)
            nc.vector.memset(acc1[:], 0.0)
            with tc.tile_pool(name="gp", bufs=24) as gp, tc.tile_pool(
                name="gpi", bufs=64
            ) as gpi:
                passes = []
                for j in range(maxd1):
                    passes.append((int(col_l1e[j]), int(C1[j]), etab))
                for j in range(maxd1):
                    passes.append((int(col_l1n[j]), int(C1[j]), agN_out))
                for c0, nch, table in passes:
                    for ch in range(nch):
                        ix1 = gpi.tile([128, 1], u32, tag="ix1")
                        nc.vector.tensor_copy(ix1[:], idxs[:, c0 + ch : c0 + ch + 1])
                        stg = gp.tile([128, 64], bf16, tag="stg")
                        nc.gpsimd.indirect_dma_start(
                            out=stg[:, :],
                            out_offset=None,
                            in_=table[:, :],
                            in_offset=bass.IndirectOffsetOnAxis(ap=ix1[:, :], axis=0),
                        )
                        nc.vector.tensor_tensor(
                            acc1[:, ch, :], acc1[:, ch, :], stg[:], op=AX.add
                        )

                # ---- h = relu(root1 + acc1); hT via PE transposes ----
                for ch in range(NCH):
                    sl = slice(ch * 128, (ch + 1) * 128)
                    p = psC.tile([128, 64], fp32, space="PSUM", tag="pc")
                    nc.tensor.matmul(
                        p[:], lhsT=nT[:, sl], rhs=wroot1[:], start=True, stop=True
                    )
                    nc.vector.tensor_tensor(p[:], p[:], acc1[:, ch, :], op=AX.add)
                    hch = wk.tile([128, 64], bf16, tag="hch")
                    nc.vector.tensor_scalar_max(hch[:], p[:], 0.0)
                    pt = psC.tile([64, 128], bf16, space="PSUM", tag="pc")
                    nc.tensor.transpose(pt[:], hch[:], ident[:])
                    nc.scalar.activation(
                        hT[0:64, sl], pt[:], mybir.ActivationFunctionType.Copy
                    )

            # ---- H-table + AllGather ----
            for ch in range(NCH):
                sl = slice(ch * 128, (ch + 1) * 128)
                p = psB.tile([128, R * 64], fp32, space="PSUM", tag="p")
                nc.tensor.matmul(
                    p[:], lhsT=hT[0:64, sl], rhs=w2a[:], start=True, stop=True
                )
                stagH = wk.tile([128, R, 64], bf16, tag="stagN")
                nc.scalar.activation(
                    stagH[:].rearrange("p r f -> p (r f)"),
                    p[:],
                    mybir.ActivationFunctionType.Copy,
                )
                nc.sync.dma_start(
                    out=agH_in[0 : R * NSP, :]
                    .rearrange("(r ch p) f -> ch p r f", p=128, ch=NCH)[ch],
                    in_=stagH[:],
                )
            nc.sync.dma_start(out=agH_in[ZROW : ZROW + 1, :], in_=zrow[:])
            nc.sync.dma_start(out=agH_in[NROW : NROW + 1, :], in_=nrow[:])
            nc.gpsimd.collective_compute(
                "AllGather",
                mybir.AluOpType.bypass,
                replica_groups=groups,
                ins=[agH_in[:, :]],
                outs=[agH_out[:, :]],
            )

            # ---- layer-2 max: per-chunk staged gathers + DVE max folds (slabs) ----
            L2SLAB = 226
            nslabs = _ceil(CH2, L2SLAB)
            with tc.tile_pool(name="l2p", bufs=1) as l2p, tc.tile_pool(
                name="gp2", bufs=24
            ) as gp2, tc.tile_pool(name="gpi2", bufs=64) as gpi2:
                for sb in range(nslabs):
                    sch0 = sb * L2SLAB
                    sch1 = min(CH2, sch0 + L2SLAB)
                    a2s = l2p.tile([128, L2SLAB, 64], bf16, tag="a2s")
                    nc.vector.memset(a2s[:], NEGBIG)
                    for j in range(maxd2):
                        c0 = int(col_l2[j])
                        hi = min(int(C2[j]), sch1)
                        for ch in range(sch0, hi):
                            ix1 = gpi2.tile([128, 1], u32, tag="ix1")
                            nc.vector.tensor_copy(
                                ix1[:], idxs[:, c0 + ch : c0 + ch + 1]
                            )
                            stg = gp2.tile([128, 64], bf16, tag="stg")
                            nc.gpsimd.indirect_dma_start(
                                out=stg[:, :],
                                out_offset=None,
                                in_=agH_out[:, :],
                                in_offset=bass.IndirectOffsetOnAxis(
                                    ap=ix1[:, :], axis=0
                                ),
                            )
                            nc.vector.tensor_tensor(
                                a2s[:, ch - sch0, :],
                                a2s[:, ch - sch0, :],
                                stg[:],
                                op=AX.max,
                            )
                    nc.sync.dma_start(
                        out=s2buf[sch0 * 128 : sch1 * 128, :].rearrange(
                            "(ch p) f -> p ch f", p=128
                        ),
                        in_=a2s[:, 0 : sch1 - sch0, :],
                    )
            nc.sync.dma_start(out=s2buf[CH2 * 128 : CH2 * 128 + 1, :], in_=zrow[:])

            # ---- stage2: per-node sum of its nonempty-rel maxes ----
            acc2e = pp.tile([128, NCH, 64], bf16)
            nc.vector.memset(acc2e[:], 0.0)
            with tc.tile_pool(name="gp3", bufs=16) as gp3, tc.tile_pool(
                name="gpi3", bufs=64
            ) as gpi3:
                for k in range(8):
                    c0 = int(col_s2[k])
                    for ch in range(NCH):
                        ix1 = gpi3.tile([128, 1], u32, tag="ix1")
                        nc.vector.tensor_copy(ix1[:], idxs[:, c0 + ch : c0 + ch + 1])
                        stg = gp3.tile([128, 64], bf16, tag="stg")
                        nc.gpsimd.indirect_dma_start(
                            out=stg[:, :],
                            out_offset=None,
                            in_=s2buf[:, :],
                            in_offset=bass.IndirectOffsetOnAxis(ap=ix1[:, :], axis=0),
                        )
                        nc.vector.tensor_tensor(
                            acc2e[:, ch, :], acc2e[:, ch, :], stg[:], op=AX.add
                        )

            # ---- h2 = relu(root2 + acc2e); head ----
            y = pp.tile([128, NCH], fp32)
            for ch in range(NCH):
                sl = slice(ch * 128, (ch + 1) * 128)
                p = psC.tile([128, 64], fp32, space="PSUM", tag="pc")
                nc.tensor.matmul(
                    p[:], lhsT=hT[:, sl], rhs=wroot2[:], start=True, stop=True
                )
                nc.vector.tensor_tensor(p[:], p[:], acc2e[:, ch, :], op=AX.add)
                h2 = wk.tile([128, 64], bf16, tag="h2")
                nc.vector.tensor_scalar_max(h2[:], p[:], 0.0)
                pt = psC.tile([64, 128], bf16, space="PSUM", tag="pc")
                nc.tensor.transpose(pt[:], h2[:], ident[:])
                h2T = wk.tile([65, 128], bf16, tag="h2T")
                nc.scalar.activation(
                    h2T[0:64, :], pt[:], mybir.ActivationFunctionType.Copy
                )
                nc.vector.memset(h2T[64:65, :], 1.0)
                omch = wk.tile([3, 128], fp32, tag="omch")
                nc.sync.dma_start(out=omch[:], in_=omT[:, sl])
                po = psC.tile([64, 128], fp32, space="PSUM", tag="pc")
                nc.tensor.matmul(po[:], lhsT=wox[:], rhs=omch[:], start=True, stop=True)
                oTc = wk.tile([64, 128], bf16, tag="oTc")
                nc.scalar.activation(oTc[:], po[:], mybir.ActivationFunctionType.Relu)
                p3 = psC.tile([128, 64], fp32, space="PSUM", tag="pc")
                nc.tensor.matmul(p3[:], lhsT=h2T[:], rhs=wagga[:], start=True, stop=False)
                nc.tensor.matmul(
                    p3[:], lhsT=oTc[:], rhs=waggb[:], start=False, stop=True
                )
                h3 = wk.tile([128, 64], bf16, tag="h3")
                nc.vector.tensor_scalar_max(h3[:], p3[:], 0.0)
                pt2 = psC.tile([64, 128], bf16, space="PSUM", tag="pc")
                nc.tensor.transpose(pt2[:], h3[:], ident[:])
                h3T = wk.tile([65, 128], bf16, tag="h3T")
                nc.scalar.activation(
                    h3T[0:64, :], pt2[:], mybir.ActivationFunctionType.Copy
                )
                nc.vector.memset(h3T[64:65, :], 1.0)
                py = psC.tile([128, 64], fp32, space="PSUM", tag="pc")
                nc.tensor.matmul(py[:], lhsT=h3T[:], rhs=wcx[:], start=True, stop=True)
                nc.scalar.activation(
                    y[:, ch : ch + 1],
                    py[:, 0:1],
                    mybir.ActivationFunctionType.Tanh,
                )
            nc.vector.tensor_scalar_mul(y[:], y[:], 5.0)
            nc.sync.dma_start(out=out_d[:, :], in_=y[:])

    nc.compile()
    return nc


def _set_sizes(n, nsp):
    """Shrink problem sizes for simulator tests."""
    global N, NS, NSP, NCH, SEG, ZROW, NROW
    N = n
    NS = n // NCORES
    NSP = nsp
    NCH = NSP // 128
    SEG = R * NSP + 2
    ZROW = R * NSP
    NROW = R * NSP + 1


# ---------------------------------------------------------------------------
# entry point
# ---------------------------------------------------------------------------

_CACHE = {}
LAST_RUN_SECONDS = None


def kernel(**inputs):
    import time
    from concourse.bass_utils import run_bass_kernel_spmd

    global LAST_RUN_SECONDS
    edge_index = np.asarray(inputs["edge_index"])
    edge_type = np.asarray(inputs["edge_type"])

    import hashlib

    key = hashlib.md5(edge_index.tobytes() + edge_type.tobytes()).hexdigest()
    if key not in _CACHE:
        cores, common = preprocess(edge_index, edge_type)
        nc = build_graph(common)
        _CACHE[key] = (cores, common, nc)
    cores, common, nc = _CACHE[key]
    in_maps = build_core_inputs(inputs, cores, common)

    t0 = time.time()
    res = run_bass_kernel_spmd(nc, in_maps, core_ids=list(range(NCORES)))
    LAST_RUN_SECONDS = time.time() - t0

    out = np.empty((N, 1), np.float32)
    for c in range(NCORES):
        o = res.results[c]["out"]  # [128, NCH]
        ranks = cores[c]["rank"]  # node -> position
        out[c * NS : (c + 1) * NS, 0] = o[ranks % 128, ranks // 128]
    return out


if __name__ == "__main__":
    import reference

    inputs = reference.setup_inputs()
    expected = np.asarray(reference.reference(**inputs))
    got = kernel(**{k: np.asarray(v) for k, v in inputs.items()})
    rel = np.linalg.norm(got - expected) / np.linalg.norm(expected)
    print(f"Relative error: {rel:.3e}")

